# revision 15
# baseline (speedup 1.0000x reference)
"""Trainium2 Bass kernel for BatchWiseTripletDistanceLoss.

Math: loss = mean_t relu(cos_d(s[a_t], s[p_t]) - cos_d(s[a_t], s[n_t]) + margin)
with cos_d(x, y) = 1 - <x,y>/max(|x||y|, eps).  The "1-" cancels in the
difference, so with C[q, j] = <s_q, s_j>/(|s_q||s_j|) each triplet term is
relu(C[a,n] - C[a,p] + margin).

Device algorithm (per core; grid = 128 anchor rows x 256 negative columns):
  - sim = (R S) (S^T R') via TensorE on an f16 copy of samples whose column
    order is a per-core permutation `perm` placing the core's own 128 rows
    at positions 192:320 (so the matmul's stationary operand and the row
    norms are slices of the same tensor) and the core's negative half at
    positions 0:256.
  - Triplets of each row are bucketed host-side, sorted by positive id, so
    equal-positive triplets form contiguous runs of slots.  A single gpsimd
    local_scatter writes both halves of a [128, 512] grid: slots 0:256
    (`buk`) get C[a,n] per triplet slot, slots 256:512 (`vgrid`) get C[a,p]
    at each run start.  A DVE prefix scan
        state = keep * state - vgrid     (keep = 0 at run starts, 1 inside)
    forward-fills -C[a,p] across each run; a sentinel value +BIG scattered
    just past the last slot poisons the empty tail.
  - loss terms: relu(z + margin) = max(z, -margin) + margin, so one DVE
    tensor_tensor_reduce(max, add-reduce) of z = buk + scan against -margin
    yields per-partition sums; the host adds the 256*margin constant back
    (empty slots hit the sentinel and contribute exactly 0).

Host does layout/indexing only (permutations, bucketing, run starts);
all floating-point math runs on device.

Sharding: 8 cores = (anchor row mod 4) x (negative column half).
"""
import sys

sys.path.insert(0, "/opt/trn_rl_repo")

from contextlib import ExitStack

import numpy as np

import concourse.bacc as bacc
import concourse.bass as bass
import concourse.tile as tile
from concourse import mybir
from concourse.bass_utils import run_bass_kernel_spmd

DT = mybir.dt
OP = mybir.AluOpType
ACTF = mybir.ActivationFunctionType

N = 512
D = 256
MARGIN = 0.15
NCORES = 8
NROW = 128  # anchor rows per core
LCOL = 256  # negative columns per core
BIG = 60000.0  # f16-safe sentinel
WPB = 256 + 514 + 256  # nidx | sidx2 | keep


def _build_program():
    nc = bacc.Bacc(
        "TRN2", target_bir_lowering=False, debug=False, num_devices=NCORES
    )
    f32, i16, f16 = DT.float32, DT.int16, DT.float16

    d_packa = nc.dram_tensor("packa", [128, 1024], f16, kind="ExternalInput").ap()
    d_packb = nc.dram_tensor("packb", [NROW, WPB], i16, kind="ExternalInput").ap()
    d_out = nc.dram_tensor("out", [NROW, 1], f32, kind="ExternalOutput").ap()

    with tile.TileContext(nc) as tc, ExitStack() as ctx:
        cpool = ctx.enter_context(tc.tile_pool(name="const", bufs=1))
        wpool = ctx.enter_context(tc.tile_pool(name="work", bufs=2))
        ppool = ctx.enter_context(tc.tile_pool(name="psum", bufs=2, space="PSUM"))
        pbig = ctx.enter_context(tc.tile_pool(name="psumbig", bufs=1, space="PSUM"))

        # ---- inputs (three DMAs on separate queues) ---------------------
        st = cpool.tile([128, 1024], f16)
        nc.sync.dma_start(st[:, 0:512], d_packa[:, 0:512])
        nc.scalar.dma_start(st[:, 512:1024], d_packa[:, 512:1024])
        pb = cpool.tile([NROW, WPB], i16)
        nc.sync.dma_start(pb[:], d_packb)
        scidx = pb[:, 0:770]
        keepg = pb[:, 770:1026].bitcast(f16)

        ones_col = cpool.tile([128, 1], f16)
        nc.vector.memset(ones_col[:], 1.0)
        ones_row = cpool.tile([1, 128], f16)
        nc.vector.memset(ones_row[:], 1.0)
        one_mov = cpool.tile([1, 1], f16)
        nc.vector.memset(one_mov[:], 1.0)
        mbias = cpool.tile([128, 1], f32)
        nc.vector.memset(mbias[:], MARGIN)

        # preload the rsqrt ACT table during the DMA dead time
        import os

        simsafe = bool(os.environ.get("K_SIMSAFE"))
        actf_rsqrt = ACTF.Sqrt if simsafe else ACTF.Abs_reciprocal_sqrt
        dumin = cpool.tile([1, 1], f32)
        nc.vector.memset(dumin[:], 4.0)
        dum1 = cpool.tile([1, 1], f32)
        nc.scalar.activation(dum1[:], dumin[:], actf_rsqrt)

        # ---- squares -> column norms (pipelined per DMA half) -----------
        sq = wpool.tile([128, 1024], f16, tag="sq")
        for k in range(2):
            ks = slice(512 * k, 512 * k + 512)
            nc.vector.tensor_tensor(sq[:, ks], st[:, ks], st[:, ks], OP.mult)
        n2p = ppool.tile([1, N], f32, tag="n2p")
        for k in range(2):
            nc.tensor.matmul(
                n2p[:], ones_col[:], sq[:, 512 * k : 512 * k + 512],
                start=(k == 0), stop=(k == 1),
            )
        rrow16 = wpool.tile([1, N], f16, tag="rrow16")
        if simsafe:
            nrow = wpool.tile([1, N], f32, tag="nrow")
            nc.scalar.activation(nrow[:], n2p[:], ACTF.Sqrt)
            rrowf = wpool.tile([1, N], f32, tag="rrowf")
            rscr = wpool.tile([1, N], f32, tag="rscr")
            nc.vector.reciprocal_approx_accurate(rrowf[:], nrow[:], rscr[:])
            nc.vector.tensor_scalar(rrow16[:], rrowf[:], 1.0, 0.0, OP.mult, OP.add)
        else:
            nc.scalar.activation(rrow16[:], n2p[:], ACTF.Abs_reciprocal_sqrt)

        # ---- sim matrix (own rows x all columns) ------------------------
        simp = pbig.tile([128, N], f32, tag="simp")
        for k in range(2):
            nc.tensor.matmul(
                simp[:], st[:, 512 * k + 192 : 512 * k + 320],
                st[:, 512 * k : 512 * k + 512],
                start=(k == 0), stop=(k == 1),
            )

        # rbp[q, j] = rrow[j]; rrp[q, 0] = rrow[192 + q] (own-row rsqrt)
        rbp = pbig.tile([128, N], f32, tag="rbp")
        nc.tensor.matmul(rbp[:], ones_row[:], rrow16[:], start=True, stop=True)
        rrp = ppool.tile([128, 1], f32, tag="rrp")
        nc.tensor.matmul(rrp[:], rrow16[0:1, 192:320], one_mov[:], start=True, stop=True)
        rr_sb = cpool.tile([128, 1], f32)
        nc.vector.tensor_scalar(rr_sb[:], rrp[:], 1.0, 0.0, OP.mult, OP.add)

        # ---- C16 = rr * simp * rrow (cosine sim, f16) -------------------
        # layout: [0:512] C, [512:514] +BIG sentinels, [514:770] dup of C[:, 0:256]
        C16 = cpool.tile([128, 770], f16)
        nc.vector.memset(C16[:, 512:514], BIG)
        t0 = wpool.tile([128, N], f32, tag="t0")
        for h in range(2):
            cs = slice(256 * h, 256 * h + 256)
            nc.scalar.activation(t0[:, cs], simp[:, cs], ACTF.Copy, scale=rr_sb[:])
            nc.vector.tensor_tensor(C16[:, cs], t0[:, cs], rbp[:, cs], OP.mult)
        nc.scalar.copy(C16[:, 514:770], C16[:, 0:256])

        # ---- one combined bucket scatter (gpsimd) -----------------------
        # grid[:, 0:256] = buk (C[a,n] per slot), grid[:, 256:512] = vgrid
        # (C[a,p] at run starts, +BIG sentinel past the last slot)
        grid = wpool.tile([NROW, 512], f16, tag="grid")
        nc.gpsimd.local_scatter(
            grid[:], C16[:], scidx, channels=128, num_elems=512, num_idxs=770
        )

        # ---- forward-fill -C[a,p] across runs, add, relu, accumulate ----
        biasg = wpool.tile([NROW, LCOL], f32, tag="biasg")
        nc.vector.tensor_tensor_scan(
            biasg[:], keepg, grid[:, 256:512], 0.0, OP.mult, OP.subtract
        )
        y = wpool.tile([NROW, LCOL], f32, tag="y")
        nc.vector.tensor_tensor(y[:], grid[:, 0:256], biasg[:], OP.add)
        acc = wpool.tile([NROW, 1], f32, tag="acc")
        rl = wpool.tile([NROW, LCOL], f32, tag="rl")
        nc.scalar.activation(
            rl[:], y[:], ACTF.Relu, bias=mbias[:], accum_out=acc[:]
        )
        nc.sync.dma_start(d_out, acc[:])

    nc.compile()
    return nc


_PROGRAM = None


def _get_program():
    global _PROGRAM
    if _PROGRAM is None:
        _PROGRAM = _build_program()
    return _PROGRAM


def _shard_inputs(samples, a, p, n):
    """Per-core layout: permute samples, bucket triplets (sorted by positive
    id so equal-positive slots are contiguous runs), build scatter indices."""
    in_maps = []
    allr = np.arange(N, dtype=np.int64)
    for core in range(NCORES):
        R, H = core >> 1, core & 1
        inH = (allr >> 8) == H
        ownm = (allr & 3) == R
        own_H = allr[ownm & inH]          # 64
        own_O = allr[ownm & ~inH]         # 64
        non_own_H = allr[~ownm & inH]     # 192
        non_own_O = allr[~ownm & ~inH]    # 192
        perm = np.concatenate([non_own_H, own_H, own_O, non_own_O])
        colpos = np.empty(N, dtype=np.int64)
        colpos[perm] = np.arange(N)
        rows_core = np.concatenate([own_H, own_O])  # partition q -> global row
        qof = np.full(N, -1, dtype=np.int64)
        qof[rows_core] = np.arange(NROW)

        sel = ((a & 3) == R) & ((n >> 8) == H)
        asel, psel, nsel = a[sel], p[sel], n[sel]
        q = qof[asel]
        order = np.lexsort((psel, q))
        qs, ps, ns = q[order], psel[order], nsel[order]
        counts = np.bincount(qs, minlength=NROW)
        starts = np.zeros(NROW, dtype=np.int64)
        starts[1:] = np.cumsum(counts)[:-1]
        slot = np.arange(len(qs)) - starts[qs]  # slot within row (sorted by p)

        # combined scatter index: data cols 0:514 = C (vgrid targets 256+),
        # data cols 514:770 = dup of C[:, 0:256] (buk targets 0:256)
        scidx = np.full((NROW, 770), -1, dtype=np.int16)
        if len(qs):
            newrun = np.ones(len(qs), dtype=bool)
            newrun[1:] = (qs[1:] != qs[:-1]) | (ps[1:] != ps[:-1])
        else:
            newrun = np.zeros(0, dtype=bool)
        scidx[qs[newrun], colpos[ps[newrun]]] = (256 + slot[newrun]).astype(np.int16)
        has_room = counts < 256
        scidx[has_room, 512] = (256 + counts[has_room]).astype(np.int16)
        scidx[qs, 514 + colpos[ns]] = slot.astype(np.int16)

        keep = np.ones((NROW, 256), dtype=np.float16)
        keep[qs[newrun], slot[newrun]] = 0.0
        keep[has_room, np.minimum(counts, 255)[has_room]] = 0.0

        A16 = samples[perm].astype(np.float16)  # [512, 256]
        AT = np.ascontiguousarray(A16.T)  # [256, 512] = (d, col)
        packa = np.concatenate([AT[0:128], AT[128:256]], axis=1)  # [128, 1024]
        packb = np.concatenate([scidx, keep.view(np.int16)], axis=1)
        in_maps.append({"packa": packa, "packb": packb})
    return in_maps


def kernel(samples, targets, anchor_idx, pos_idx, neg_idx, _want_trace=False):
    samples = np.asarray(samples, dtype=np.float32)
    a = np.asarray(anchor_idx).astype(np.int64)
    p = np.asarray(pos_idx).astype(np.int64)
    n = np.asarray(neg_idx).astype(np.int64)
    T = a.shape[0]
    assert samples.shape == (N, D)

    ok = (
        np.all((a >= 0) & (a < N) & (p >= 0) & (p < N) & (n >= 0) & (n < N))
        and len(np.unique(a * N + n)) == T
    )
    if not ok:
        raise NotImplementedError("inputs violate mined-triplet structure")

    nc = _get_program()
    in_maps = _shard_inputs(samples, a, p, n)
    res = run_bass_kernel_spmd(nc, in_maps, list(range(NCORES)), trace=_want_trace)
    total = sum(
        float(res.results[c]["out"].astype(np.float64).sum()) for c in range(NCORES)
    )
    loss = np.float32(total / T)
    if _want_trace:
        return loss, res
    return loss


# revision 16
# speedup vs baseline: 1.1196x; 1.1196x over previous
"""Trainium2 Bass kernel for BatchWiseTripletDistanceLoss.

Math: loss = mean_t relu(cos_d(s[a_t], s[p_t]) - cos_d(s[a_t], s[n_t]) + margin)
with cos_d(x, y) = 1 - <x,y>/max(|x||y|, eps).  The "1-" cancels in the
difference, so with C[q, j] = <s_q, s_j>/(|s_q||s_j|) each triplet term is
relu(C[a,n] - C[a,p] + margin).

Device algorithm (per core; grid = 128 anchor rows x 256 negative columns):
  - sim = (R S) (S^T R') via TensorE on an f16 copy of samples whose column
    order is a per-core permutation `perm` placing the core's own 128 rows
    at positions 192:320 (so the matmul's stationary operand and the row
    norms are slices of the same tensor) and the core's negative half at
    positions 0:256.
  - Triplets of each row are bucketed host-side, sorted by positive id, so
    equal-positive triplets form contiguous runs of slots.  A single gpsimd
    local_scatter writes both halves of a [128, 512] grid: slots 0:256
    (`buk`) get C[a,n] per triplet slot, slots 256:512 (`vgrid`) get C[a,p]
    at each run start.  A DVE prefix scan
        state = keep * state - vgrid     (keep = 0 at run starts, 1 inside)
    forward-fills -C[a,p] across each run; a sentinel value +BIG scattered
    just past the last slot poisons the empty tail.
  - loss terms: relu(z + margin) = max(z, -margin) + margin, so one DVE
    tensor_tensor_reduce(max, add-reduce) of z = buk + scan against -margin
    yields per-partition sums; the host adds the 256*margin constant back
    (empty slots hit the sentinel and contribute exactly 0).

Host does layout/indexing only (permutations, bucketing, run starts);
all floating-point math runs on device.

Sharding: 8 cores = (anchor row mod 4) x (negative column half).
"""
import sys

sys.path.insert(0, "/opt/trn_rl_repo")

from contextlib import ExitStack

import numpy as np

import concourse.bacc as bacc
import concourse.bass as bass
import concourse.tile as tile
from concourse import mybir
from concourse.bass_utils import run_bass_kernel_spmd

DT = mybir.dt
OP = mybir.AluOpType
ACTF = mybir.ActivationFunctionType

N = 512
D = 256
MARGIN = 0.15
NCORES = 8
NROW = 128  # anchor rows per core
LCOL = 256  # negative columns per core
BIG = 60000.0  # f16-safe sentinel
WPB = 256 + 514 + 256  # nidx | sidx2 | keep


def _build_program():
    nc = bacc.Bacc(
        "TRN2", target_bir_lowering=False, debug=False, num_devices=NCORES
    )
    f32, i16, f16 = DT.float32, DT.int16, DT.float16

    d_packa = nc.dram_tensor("packa", [128, 1024], f16, kind="ExternalInput").ap()
    d_packb = nc.dram_tensor("packb", [NROW, WPB], i16, kind="ExternalInput").ap()
    d_out = nc.dram_tensor("out", [NROW, 1], f32, kind="ExternalOutput").ap()

    with tile.TileContext(nc) as tc, ExitStack() as ctx:
        cpool = ctx.enter_context(tc.tile_pool(name="const", bufs=1))
        wpool = ctx.enter_context(tc.tile_pool(name="work", bufs=2))
        ppool = ctx.enter_context(tc.tile_pool(name="psum", bufs=2, space="PSUM"))
        pbig = ctx.enter_context(tc.tile_pool(name="psumbig", bufs=1, space="PSUM"))

        # ---- inputs (three DMAs on separate queues) ---------------------
        st = cpool.tile([128, 1024], f16)
        nc.sync.dma_start(st[:, 0:512], d_packa[:, 0:512])
        nc.scalar.dma_start(st[:, 512:1024], d_packa[:, 512:1024])
        pb = cpool.tile([NROW, WPB], i16)
        nc.sync.dma_start(pb[:], d_packb)
        nidx = pb[:, 0:256]
        sidx2 = pb[:, 256:770]
        keepg = pb[:, 770:1026].bitcast(f16)

        ones_col = cpool.tile([128, 1], f16)
        nc.vector.memset(ones_col[:], 1.0)
        ones_row = cpool.tile([1, 128], f16)
        nc.vector.memset(ones_row[:], 1.0)
        one_mov = cpool.tile([1, 1], f16)
        nc.vector.memset(one_mov[:], 1.0)
        mbias = cpool.tile([128, 1], f32)
        nc.vector.memset(mbias[:], MARGIN)

        # preload the rsqrt ACT table during the DMA dead time
        import os

        simsafe = bool(os.environ.get("K_SIMSAFE"))
        actf_rsqrt = ACTF.Sqrt if simsafe else ACTF.Abs_reciprocal_sqrt
        dumin = cpool.tile([1, 1], f32)
        nc.vector.memset(dumin[:], 4.0)
        dum1 = cpool.tile([1, 1], f32)
        nc.scalar.activation(dum1[:], dumin[:], actf_rsqrt)

        # ---- squares -> column norms (pipelined per DMA half) -----------
        sq = wpool.tile([128, 1024], f16, tag="sq")
        for k in range(2):
            ks = slice(512 * k, 512 * k + 512)
            nc.vector.tensor_tensor(sq[:, ks], st[:, ks], st[:, ks], OP.mult)
        n2p = ppool.tile([1, N], f32, tag="n2p")
        for k in range(2):
            nc.tensor.matmul(
                n2p[:], ones_col[:], sq[:, 512 * k : 512 * k + 512],
                start=(k == 0), stop=(k == 1),
            )
        rrow16 = wpool.tile([1, N], f16, tag="rrow16")
        if simsafe:
            nrow = wpool.tile([1, N], f32, tag="nrow")
            nc.scalar.activation(nrow[:], n2p[:], ACTF.Sqrt)
            rrowf = wpool.tile([1, N], f32, tag="rrowf")
            rscr = wpool.tile([1, N], f32, tag="rscr")
            nc.vector.reciprocal_approx_accurate(rrowf[:], nrow[:], rscr[:])
            nc.vector.tensor_scalar(rrow16[:], rrowf[:], 1.0, 0.0, OP.mult, OP.add)
        else:
            nc.scalar.activation(rrow16[:], n2p[:], ACTF.Abs_reciprocal_sqrt)

        # ---- sim matrix (own rows x all columns) ------------------------
        simp = pbig.tile([128, N], f32, tag="simp")
        for k in range(2):
            nc.tensor.matmul(
                simp[:], st[:, 512 * k + 192 : 512 * k + 320],
                st[:, 512 * k : 512 * k + 512],
                start=(k == 0), stop=(k == 1),
            )

        # rbp[q, j] = rrow[j]; rrp[q, 0] = rrow[192 + q] (own-row rsqrt)
        rrp = ppool.tile([128, 1], f32, tag="rrp")
        nc.tensor.matmul(rrp[:], rrow16[0:1, 192:320], one_mov[:], start=True, stop=True)
        rr_sb = cpool.tile([128, 1], f32)
        nc.vector.tensor_scalar(rr_sb[:], rrp[:], 1.0, 0.0, OP.mult, OP.add)
        rbp = pbig.tile([128, N], f32, tag="rbp")
        nc.tensor.matmul(rbp[:], ones_row[:], rrow16[:], start=True, stop=True)

        # ---- C16 = rr * simp * rrow  (cosine sim, f16, + sentinel cols) --
        C16 = cpool.tile([128, 514], f16)
        nc.vector.memset(C16[:, 512:514], BIG)
        t0 = wpool.tile([128, N], f32, tag="t0")
        for h in range(2):
            cs = slice(256 * h, 256 * h + 256)
            nc.scalar.activation(t0[:, cs], simp[:, cs], ACTF.Copy, scale=rr_sb[:])
            nc.vector.tensor_tensor(C16[:, cs], t0[:, cs], rbp[:, cs], OP.mult)

        # ---- bucket scatters (gpsimd) -----------------------------------
        buk = wpool.tile([NROW, LCOL], f16, tag="buk")
        nc.gpsimd.local_scatter(
            buk[:], C16[:, 0:256], nidx, channels=128, num_elems=LCOL, num_idxs=256
        )
        vgrid = wpool.tile([NROW, LCOL], f16, tag="vgrid")
        nc.gpsimd.local_scatter(
            vgrid[:], C16[:], sidx2, channels=128, num_elems=LCOL, num_idxs=514
        )

        # ---- forward-fill -C[a,p] across runs, add, relu, accumulate ----
        biasg = wpool.tile([NROW, LCOL], f32, tag="biasg")
        nc.vector.tensor_tensor_scan(
            biasg[:], keepg, vgrid[:], 0.0, OP.mult, OP.subtract
        )
        y = wpool.tile([NROW, LCOL], f32, tag="y")
        nc.vector.tensor_tensor(y[:], buk[:], biasg[:], OP.add)
        acc = wpool.tile([NROW, 1], f32, tag="acc")
        rl = wpool.tile([NROW, LCOL], f32, tag="rl")
        nc.scalar.activation(
            rl[:], y[:], ACTF.Relu, bias=mbias[:], accum_out=acc[:]
        )
        nc.sync.dma_start(d_out, acc[:])

    nc.compile()
    return nc


_PROGRAM = None


def _get_program():
    global _PROGRAM
    if _PROGRAM is None:
        _PROGRAM = _build_program()
    return _PROGRAM


def _shard_inputs(samples, a, p, n):
    """Per-core layout: permute samples, bucket triplets (sorted by positive
    id so equal-positive slots are contiguous runs), build scatter indices."""
    in_maps = []
    allr = np.arange(N, dtype=np.int64)
    for core in range(NCORES):
        R, H = core >> 1, core & 1
        inH = (allr >> 8) == H
        ownm = (allr & 3) == R
        own_H = allr[ownm & inH]          # 64
        own_O = allr[ownm & ~inH]         # 64
        non_own_H = allr[~ownm & inH]     # 192
        non_own_O = allr[~ownm & ~inH]    # 192
        perm = np.concatenate([non_own_H, own_H, own_O, non_own_O])
        colpos = np.empty(N, dtype=np.int64)
        colpos[perm] = np.arange(N)
        rows_core = np.concatenate([own_H, own_O])  # partition q -> global row
        qof = np.full(N, -1, dtype=np.int64)
        qof[rows_core] = np.arange(NROW)

        sel = ((a & 3) == R) & ((n >> 8) == H)
        asel, psel, nsel = a[sel], p[sel], n[sel]
        q = qof[asel]
        order = np.lexsort((psel, q))
        qs, ps, ns = q[order], psel[order], nsel[order]
        counts = np.bincount(qs, minlength=NROW)
        starts = np.zeros(NROW, dtype=np.int64)
        starts[1:] = np.cumsum(counts)[:-1]
        slot = np.arange(len(qs)) - starts[qs]  # slot within row (sorted by p)

        nidx = np.full((NROW, 256), -1, dtype=np.int16)
        nidx[qs, colpos[ns]] = slot.astype(np.int16)
        if len(qs):
            newrun = np.ones(len(qs), dtype=bool)
            newrun[1:] = (qs[1:] != qs[:-1]) | (ps[1:] != ps[:-1])
        else:
            newrun = np.zeros(0, dtype=bool)
        sidx2 = np.full((NROW, 514), -1, dtype=np.int16)
        sidx2[qs[newrun], colpos[ps[newrun]]] = slot[newrun].astype(np.int16)
        has_room = counts < 256
        sidx2[has_room, 512] = counts[has_room].astype(np.int16)

        keep = np.ones((NROW, 256), dtype=np.float16)
        keep[qs[newrun], slot[newrun]] = 0.0
        keep[has_room, np.minimum(counts, 255)[has_room]] = 0.0

        A16 = samples[perm].astype(np.float16)  # [512, 256]
        AT = np.ascontiguousarray(A16.T)  # [256, 512] = (d, col)
        packa = np.concatenate([AT[0:128], AT[128:256]], axis=1)  # [128, 1024]
        packb = np.concatenate([nidx, sidx2, keep.view(np.int16)], axis=1)
        in_maps.append({"packa": packa, "packb": packb})
    return in_maps


def kernel(samples, targets, anchor_idx, pos_idx, neg_idx, _want_trace=False):
    samples = np.asarray(samples, dtype=np.float32)
    a = np.asarray(anchor_idx).astype(np.int64)
    p = np.asarray(pos_idx).astype(np.int64)
    n = np.asarray(neg_idx).astype(np.int64)
    T = a.shape[0]
    assert samples.shape == (N, D)

    ok = (
        np.all((a >= 0) & (a < N) & (p >= 0) & (p < N) & (n >= 0) & (n < N))
        and len(np.unique(a * N + n)) == T
    )
    if not ok:
        raise NotImplementedError("inputs violate mined-triplet structure")

    nc = _get_program()
    in_maps = _shard_inputs(samples, a, p, n)
    res = run_bass_kernel_spmd(nc, in_maps, list(range(NCORES)), trace=_want_trace)
    total = sum(
        float(res.results[c]["out"].astype(np.float64).sum()) for c in range(NCORES)
    )
    loss = np.float32(total / T)
    if _want_trace:
        return loss, res
    return loss


# revision 17
# speedup vs baseline: 1.3754x; 1.2284x over previous
"""Trainium2 Bass kernel for BatchWiseTripletDistanceLoss.

Math: loss = mean_t relu(cos_d(s[a_t], s[p_t]) - cos_d(s[a_t], s[n_t]) + margin)
with cos_d(x, y) = 1 - <x,y>/max(|x||y|, eps).  The "1-" cancels in the
difference, so with C[q, j] = <s_q, s_j>/(|s_q||s_j|) each triplet term is
relu(C[a,n] - C[a,p] + margin).

Device algorithm (per core; grid = 128 anchor rows x 256 negative columns):
  - sim = (R S) (S^T R') via TensorE on an f16 copy of samples whose column
    order is a per-core permutation `perm` placing the core's own 128 rows
    at positions 192:320 (so the matmul's stationary operand and the row
    norms are slices of the same tensor) and the core's negative half at
    positions 0:256.
  - Triplets of each row are bucketed host-side, sorted by positive id, so
    equal-positive triplets form contiguous runs of slots.  A single gpsimd
    local_scatter writes both halves of a [128, 512] grid: slots 0:256
    (`buk`) get C[a,n] per triplet slot, slots 256:512 (`vgrid`) get C[a,p]
    at each run start.  A DVE prefix scan
        state = keep * state - vgrid     (keep = 0 at run starts, 1 inside)
    forward-fills -C[a,p] across each run; a sentinel value +BIG scattered
    just past the last slot poisons the empty tail.
  - loss terms: relu(z + margin) = max(z, -margin) + margin, so one DVE
    tensor_tensor_reduce(max, add-reduce) of z = buk + scan against -margin
    yields per-partition sums; the host adds the 256*margin constant back
    (empty slots hit the sentinel and contribute exactly 0).

Host does layout/indexing only (permutations, bucketing, run starts);
all floating-point math runs on device.

Sharding: 8 cores = (anchor row mod 4) x (negative column half).
"""
import sys

sys.path.insert(0, "/opt/trn_rl_repo")

from contextlib import ExitStack

import numpy as np

import concourse.bacc as bacc
import concourse.bass as bass
import concourse.tile as tile
from concourse import mybir
from concourse.bass_utils import run_bass_kernel_spmd

DT = mybir.dt
OP = mybir.AluOpType
ACTF = mybir.ActivationFunctionType

N = 512
D = 256
MARGIN = 0.15
NCORES = 8
NROW = 128  # anchor rows per core
LCOL = 256  # negative columns per core
BIG = 60000.0  # f16-safe sentinel
WPB = 256 + 514 + 256  # nidx | sidx2 | keep


def _build_program():
    nc = bacc.Bacc(
        "TRN2", target_bir_lowering=False, debug=False, num_devices=NCORES
    )
    f32, i16, f16 = DT.float32, DT.int16, DT.float16

    d_packa = nc.dram_tensor("packa", [128, 1024], f16, kind="ExternalInput").ap()
    d_packb = nc.dram_tensor("packb", [NROW, WPB], i16, kind="ExternalInput").ap()
    d_out = nc.dram_tensor("out", [1, 1], f32, kind="ExternalOutput").ap()

    with tile.TileContext(nc) as tc, ExitStack() as ctx:
        cpool = ctx.enter_context(tc.tile_pool(name="const", bufs=1))
        wpool = ctx.enter_context(tc.tile_pool(name="work", bufs=2))
        ppool = ctx.enter_context(tc.tile_pool(name="psum", bufs=2, space="PSUM"))
        pbig = ctx.enter_context(tc.tile_pool(name="psumbig", bufs=1, space="PSUM"))

        # ---- inputs (three DMAs on separate queues) ---------------------
        st = cpool.tile([128, 1024], f16)
        nc.sync.dma_start(st[:, 0:512], d_packa[:, 0:512])
        nc.scalar.dma_start(st[:, 512:1024], d_packa[:, 512:1024])
        pb = cpool.tile([NROW, WPB], i16)
        nc.sync.dma_start(pb[:], d_packb)
        nidx = pb[:, 0:256]
        sidx2 = pb[:, 256:770]
        keepg = pb[:, 770:1026].bitcast(f16)

        ones_col = cpool.tile([128, 1], f16)
        nc.vector.memset(ones_col[:], 1.0)
        ones_row = cpool.tile([1, 128], f16)
        nc.vector.memset(ones_row[:], 1.0)
        ones_f32 = cpool.tile([128, 1], f32)
        nc.vector.memset(ones_f32[:], 1.0)
        mbias = cpool.tile([128, 1], f32)
        nc.vector.memset(mbias[:], MARGIN)

        # preload the rsqrt ACT table during the DMA dead time
        import os

        simsafe = bool(os.environ.get("K_SIMSAFE"))
        actf_rsqrt = ACTF.Sqrt if simsafe else ACTF.Abs_reciprocal_sqrt
        dumin = cpool.tile([1, 1], f32)
        nc.vector.memset(dumin[:], 4.0)
        dum1 = cpool.tile([1, 1], f32)
        nc.scalar.activation(dum1[:], dumin[:], actf_rsqrt)

        # ---- squares -> column norms (pipelined per DMA half) -----------
        sq = wpool.tile([128, 1024], f16, tag="sq")
        for k in range(2):
            ks = slice(512 * k, 512 * k + 512)
            nc.vector.tensor_tensor(sq[:, ks], st[:, ks], st[:, ks], OP.mult)
        n2p = ppool.tile([1, N], f32, tag="n2p")
        for k in range(2):
            nc.tensor.matmul(
                n2p[:], ones_col[:], sq[:, 512 * k : 512 * k + 512],
                start=(k == 0), stop=(k == 1),
            )
        rrow16 = wpool.tile([1, N], f16, tag="rrow16")
        if simsafe:
            nrow = wpool.tile([1, N], f32, tag="nrow")
            nc.scalar.activation(nrow[:], n2p[:], ACTF.Sqrt)
            rrowf = wpool.tile([1, N], f32, tag="rrowf")
            rscr = wpool.tile([1, N], f32, tag="rscr")
            nc.vector.reciprocal_approx_accurate(rrowf[:], nrow[:], rscr[:])
            nc.vector.tensor_scalar(rrow16[:], rrowf[:], 1.0, 0.0, OP.mult, OP.add)
        else:
            nc.scalar.activation(rrow16[:], n2p[:], ACTF.Abs_reciprocal_sqrt)

        # ---- sim matrix (own rows x all columns) ------------------------
        simp = pbig.tile([128, N], f32, tag="simp")
        for k in range(2):
            nc.tensor.matmul(
                simp[:], st[:, 512 * k + 192 : 512 * k + 320],
                st[:, 512 * k : 512 * k + 512],
                start=(k == 0), stop=(k == 1),
            )

        # rbp[q, j] = rrow[192 + q] * rrow[j]  (outer product of rsqrts)
        rbp = pbig.tile([128, N], f32, tag="rbp")
        nc.tensor.matmul(rbp[:], rrow16[0:1, 192:320], rrow16[:], start=True, stop=True)

        # ---- C16 = simp * rbp  (cosine sim, f16, + sentinel cols) -------
        C16 = cpool.tile([128, 514], f16)
        nc.vector.memset(C16[:, 512:514], BIG)
        t0 = wpool.tile([128, N], f32, tag="t0")
        for h in range(2):
            cs = slice(256 * h, 256 * h + 256)
            nc.scalar.activation(t0[:, cs], simp[:, cs], ACTF.Copy)
            nc.vector.tensor_tensor(C16[:, cs], t0[:, cs], rbp[:, cs], OP.mult)

        # ---- bucket scatters (gpsimd) -----------------------------------
        buk = wpool.tile([NROW, LCOL], f16, tag="buk")
        nc.gpsimd.local_scatter(
            buk[:], C16[:, 0:256], nidx, channels=128, num_elems=LCOL, num_idxs=256
        )
        vgrid = wpool.tile([NROW, LCOL], f16, tag="vgrid")
        nc.gpsimd.local_scatter(
            vgrid[:], C16[:], sidx2, channels=128, num_elems=LCOL, num_idxs=514
        )

        # ---- forward-fill -C[a,p] across runs, add, relu, accumulate ----
        biasg = wpool.tile([NROW, LCOL], f32, tag="biasg")
        nc.vector.tensor_tensor_scan(
            biasg[:], keepg, vgrid[:], 0.0, OP.mult, OP.subtract
        )
        y = wpool.tile([NROW, LCOL], f32, tag="y")
        nc.vector.tensor_tensor(y[:], buk[:], biasg[:], OP.add)
        acc = wpool.tile([NROW, 1], f32, tag="acc")
        rl = wpool.tile([NROW, LCOL], f32, tag="rl")
        nc.scalar.activation(
            rl[:], y[:], ACTF.Relu, bias=mbias[:], accum_out=acc[:]
        )
        totp = ppool.tile([1, 1], f32, tag="totp")
        nc.tensor.matmul(totp[:], ones_f32[:], acc[:], start=True, stop=True)
        tot = wpool.tile([1, 1], f32, tag="tot")
        nc.scalar.copy(tot[:], totp[:])
        nc.sync.dma_start(d_out, tot[:])

    nc.compile()
    return nc


_PROGRAM = None


def _get_program():
    global _PROGRAM
    if _PROGRAM is None:
        _PROGRAM = _build_program()
    return _PROGRAM


def _shard_inputs(samples, a, p, n):
    """Per-core layout: permute samples, bucket triplets (sorted by positive
    id so equal-positive slots are contiguous runs), build scatter indices."""
    in_maps = []
    allr = np.arange(N, dtype=np.int64)
    for core in range(NCORES):
        R, H = core >> 1, core & 1
        inH = (allr >> 8) == H
        ownm = (allr & 3) == R
        own_H = allr[ownm & inH]          # 64
        own_O = allr[ownm & ~inH]         # 64
        non_own_H = allr[~ownm & inH]     # 192
        non_own_O = allr[~ownm & ~inH]    # 192
        perm = np.concatenate([non_own_H, own_H, own_O, non_own_O])
        colpos = np.empty(N, dtype=np.int64)
        colpos[perm] = np.arange(N)
        rows_core = np.concatenate([own_H, own_O])  # partition q -> global row
        qof = np.full(N, -1, dtype=np.int64)
        qof[rows_core] = np.arange(NROW)

        sel = ((a & 3) == R) & ((n >> 8) == H)
        asel, psel, nsel = a[sel], p[sel], n[sel]
        q = qof[asel]
        order = np.lexsort((psel, q))
        qs, ps, ns = q[order], psel[order], nsel[order]
        counts = np.bincount(qs, minlength=NROW)
        starts = np.zeros(NROW, dtype=np.int64)
        starts[1:] = np.cumsum(counts)[:-1]
        slot = np.arange(len(qs)) - starts[qs]  # slot within row (sorted by p)

        nidx = np.full((NROW, 256), -1, dtype=np.int16)
        nidx[qs, colpos[ns]] = slot.astype(np.int16)
        if len(qs):
            newrun = np.ones(len(qs), dtype=bool)
            newrun[1:] = (qs[1:] != qs[:-1]) | (ps[1:] != ps[:-1])
        else:
            newrun = np.zeros(0, dtype=bool)
        sidx2 = np.full((NROW, 514), -1, dtype=np.int16)
        sidx2[qs[newrun], colpos[ps[newrun]]] = slot[newrun].astype(np.int16)
        has_room = counts < 256
        sidx2[has_room, 512] = counts[has_room].astype(np.int16)

        keep = np.ones((NROW, 256), dtype=np.float16)
        keep[qs[newrun], slot[newrun]] = 0.0
        keep[has_room, np.minimum(counts, 255)[has_room]] = 0.0

        A16 = samples[perm].astype(np.float16)  # [512, 256]
        AT = np.ascontiguousarray(A16.T)  # [256, 512] = (d, col)
        packa = np.concatenate([AT[0:128], AT[128:256]], axis=1)  # [128, 1024]
        packb = np.concatenate([nidx, sidx2, keep.view(np.int16)], axis=1)
        in_maps.append({"packa": packa, "packb": packb})
    return in_maps


def kernel(samples, targets, anchor_idx, pos_idx, neg_idx, _want_trace=False):
    samples = np.asarray(samples, dtype=np.float32)
    a = np.asarray(anchor_idx).astype(np.int64)
    p = np.asarray(pos_idx).astype(np.int64)
    n = np.asarray(neg_idx).astype(np.int64)
    T = a.shape[0]
    assert samples.shape == (N, D)

    ok = (
        np.all((a >= 0) & (a < N) & (p >= 0) & (p < N) & (n >= 0) & (n < N))
        and len(np.unique(a * N + n)) == T
    )
    if not ok:
        raise NotImplementedError("inputs violate mined-triplet structure")

    nc = _get_program()
    in_maps = _shard_inputs(samples, a, p, n)
    res = run_bass_kernel_spmd(nc, in_maps, list(range(NCORES)), trace=_want_trace)
    total = sum(
        float(res.results[c]["out"].astype(np.float64).sum()) for c in range(NCORES)
    )
    loss = np.float32(total / T)
    if _want_trace:
        return loss, res
    return loss


# revision 19
# speedup vs baseline: 1.4057x; 1.0220x over previous
"""Trainium2 Bass kernel for BatchWiseTripletDistanceLoss.

Math: loss = mean_t relu(cos_d(s[a_t], s[p_t]) - cos_d(s[a_t], s[n_t]) + margin)
with cos_d(x, y) = 1 - <x,y>/max(|x||y|, eps).  The "1-" cancels in the
difference, so with C[q, j] = <s_q, s_j>/(|s_q||s_j|) each triplet term is
relu(C[a,n] - C[a,p] + margin).

Device algorithm (per core; grid = 128 anchor rows x 256 negative columns):
  - sim = (R S) (S^T R') via TensorE on an f16 copy of samples whose column
    order is a per-core permutation `perm` placing the core's own 128 rows
    at positions 192:320 (so the matmul's stationary operand and the row
    norms are slices of the same tensor) and the core's negative half at
    positions 0:256.
  - Triplets of each row are bucketed host-side, sorted by positive id, so
    equal-positive triplets form contiguous runs of slots.  A single gpsimd
    local_scatter writes both halves of a [128, 512] grid: slots 0:256
    (`buk`) get C[a,n] per triplet slot, slots 256:512 (`vgrid`) get C[a,p]
    at each run start.  A DVE prefix scan
        state = keep * state - vgrid     (keep = 0 at run starts, 1 inside)
    forward-fills -C[a,p] across each run; a sentinel value +BIG scattered
    just past the last slot poisons the empty tail.
  - loss terms: relu(z + margin) = max(z, -margin) + margin, so one DVE
    tensor_tensor_reduce(max, add-reduce) of z = buk + scan against -margin
    yields per-partition sums; the host adds the 256*margin constant back
    (empty slots hit the sentinel and contribute exactly 0).

Host does layout/indexing only (permutations, bucketing, run starts);
all floating-point math runs on device.

Sharding: 8 cores = (anchor row mod 4) x (negative column half).
"""
import sys

sys.path.insert(0, "/opt/trn_rl_repo")

from contextlib import ExitStack

import numpy as np

import concourse.bacc as bacc
import concourse.bass as bass
import concourse.tile as tile
from concourse import mybir
from concourse.bass_utils import run_bass_kernel_spmd

DT = mybir.dt
OP = mybir.AluOpType
ACTF = mybir.ActivationFunctionType

N = 512
D = 256
MARGIN = 0.15
NCORES = 8
NROW = 128  # anchor rows per core
LCOL = 256  # negative columns per core
BIG = 60000.0  # f16-safe sentinel
WPB = 256 + 514 + 256  # nidx | sidx2 | keep


def _build_program():
    nc = bacc.Bacc(
        "TRN2", target_bir_lowering=False, debug=False, num_devices=NCORES
    )
    f32, i16, f16 = DT.float32, DT.int16, DT.float16

    d_packa = nc.dram_tensor("packa", [128, 1024], f16, kind="ExternalInput").ap()
    d_packb = nc.dram_tensor("packb", [NROW, WPB], i16, kind="ExternalInput").ap()
    d_out = nc.dram_tensor("out", [1, 1], f32, kind="ExternalOutput").ap()

    with tile.TileContext(nc) as tc, ExitStack() as ctx:
        cpool = ctx.enter_context(tc.tile_pool(name="const", bufs=1))
        wpool = ctx.enter_context(tc.tile_pool(name="work", bufs=2))
        ppool = ctx.enter_context(tc.tile_pool(name="psum", bufs=2, space="PSUM"))
        pbig = ctx.enter_context(tc.tile_pool(name="psumbig", bufs=1, space="PSUM"))

        # ---- inputs (three DMAs on separate queues) ---------------------
        st = cpool.tile([128, 1024], f16)
        nc.sync.dma_start(st[:, 0:512], d_packa[:, 0:512])
        nc.scalar.dma_start(st[:, 512:1024], d_packa[:, 512:1024])
        pb = cpool.tile([NROW, WPB], i16)
        nc.sync.dma_start(pb[:], d_packb)
        nidx = pb[:, 0:256]
        sidx2 = pb[:, 256:770]
        keepg = pb[:, 770:1026].bitcast(f16)

        ones_col = cpool.tile([128, 1], f16)
        nc.vector.memset(ones_col[:], 1.0)
        ones_row = cpool.tile([1, 128], f16)
        nc.vector.memset(ones_row[:], 1.0)
        ones_f32 = cpool.tile([128, 1], f32)
        nc.vector.memset(ones_f32[:], 1.0)
        mbias = cpool.tile([128, 1], f32)
        nc.vector.memset(mbias[:], MARGIN)

        # preload the rsqrt ACT table during the DMA dead time
        import os

        simsafe = bool(os.environ.get("K_SIMSAFE"))
        actf_rsqrt = ACTF.Sqrt if simsafe else ACTF.Abs_reciprocal_sqrt
        dumin = cpool.tile([1, 1], f32)
        nc.vector.memset(dumin[:], 4.0)
        dum1 = cpool.tile([1, 1], f32)
        nc.scalar.activation(dum1[:], dumin[:], actf_rsqrt)

        # ---- squares -> column norms (pipelined per DMA half) -----------
        sq = wpool.tile([128, 1024], f16, tag="sq")
        for k in range(2):
            ks = slice(512 * k, 512 * k + 512)
            nc.vector.tensor_tensor(sq[:, ks], st[:, ks], st[:, ks], OP.mult)
        n2p = ppool.tile([1, N], f32, tag="n2p")
        for k in range(2):
            nc.tensor.matmul(
                n2p[:], ones_col[:], sq[:, 512 * k : 512 * k + 512],
                start=(k == 0), stop=(k == 1),
            )
        rrow16 = wpool.tile([1, N], f16, tag="rrow16")
        if simsafe:
            nrow = wpool.tile([1, N], f32, tag="nrow")
            nc.scalar.activation(nrow[:], n2p[:], ACTF.Sqrt)
            rrowf = wpool.tile([1, N], f32, tag="rrowf")
            rscr = wpool.tile([1, N], f32, tag="rscr")
            nc.vector.reciprocal_approx_accurate(rrowf[:], nrow[:], rscr[:])
            nc.vector.tensor_scalar(rrow16[:], rrowf[:], 1.0, 0.0, OP.mult, OP.add)
        else:
            nc.scalar.activation(rrow16[:], n2p[:], ACTF.Abs_reciprocal_sqrt)

        # ---- sim matrix (own rows x all columns) ------------------------
        simp = pbig.tile([128, N], f32, tag="simp")
        for k in range(2):
            nc.tensor.matmul(
                simp[:], st[:, 512 * k + 192 : 512 * k + 320],
                st[:, 512 * k : 512 * k + 512],
                start=(k == 0), stop=(k == 1),
            )

        # rbp[q, j] = rrow[192 + q] * rrow[j]  (outer product of rsqrts)
        rbp = pbig.tile([128, N], f32, tag="rbp")
        nc.tensor.matmul(rbp[:], rrow16[0:1, 192:320], rrow16[:], start=True, stop=True)

        # ---- C16 = simp * rbp  (cosine sim, f16, + sentinel cols) -------
        C16 = cpool.tile([128, 514], f16)
        nc.vector.memset(C16[:, 512:514], BIG)
        t0 = wpool.tile([128, N], f32, tag="t0")
        for h in range(2):
            cs = slice(256 * h, 256 * h + 256)
            nc.scalar.activation(t0[:, cs], simp[:, cs], ACTF.Copy)
            nc.vector.tensor_tensor(C16[:, cs], t0[:, cs], rbp[:, cs], OP.mult)

        # ---- bucket scatters (gpsimd) -----------------------------------
        buk = wpool.tile([NROW, LCOL], f16, tag="buk")
        nc.gpsimd.local_scatter(
            buk[:], C16[:, 0:256], nidx, channels=128, num_elems=LCOL, num_idxs=256
        )
        vgrid = wpool.tile([NROW, LCOL], f16, tag="vgrid")
        nc.gpsimd.local_scatter(
            vgrid[:], C16[:], sidx2, channels=128, num_elems=LCOL, num_idxs=514
        )

        # ---- forward-fill -C[a,p] across runs, add, relu, accumulate ----
        # two chained chunks so add/relu of chunk 0 overlap the chunk-1 scan
        biasg = wpool.tile([NROW, LCOL], f32, tag="biasg")
        y = wpool.tile([NROW, LCOL], f32, tag="y")
        acc = wpool.tile([NROW, 2], f32, tag="acc")
        rl = wpool.tile([NROW, LCOL], f32, tag="rl")
        HC = LCOL // 2
        for c in range(2):
            cs = slice(HC * c, HC * c + HC)
            nc.vector.tensor_tensor_scan(
                biasg[:, cs], keepg[:, cs], vgrid[:, cs],
                0.0 if c == 0 else biasg[:, HC - 1 : HC],
                OP.mult, OP.subtract,
            )
            nc.vector.tensor_tensor(y[:, cs], buk[:, cs], biasg[:, cs], OP.add)
            nc.scalar.activation(
                rl[:, cs], y[:, cs], ACTF.Relu, bias=mbias[:],
                accum_out=acc[:, c : c + 1],
            )
        totp = ppool.tile([1, 1], f32, tag="totp")
        for c in range(2):
            nc.tensor.matmul(
                totp[:], ones_f32[:], acc[:, c : c + 1],
                start=(c == 0), stop=(c == 1),
            )
        tot = wpool.tile([1, 1], f32, tag="tot")
        nc.scalar.copy(tot[:], totp[:])
        nc.sync.dma_start(d_out, tot[:])

    nc.compile()
    return nc


_PROGRAM = None


def _get_program():
    global _PROGRAM
    if _PROGRAM is None:
        _PROGRAM = _build_program()
    return _PROGRAM


def _shard_inputs(samples, a, p, n):
    """Per-core layout: permute samples, bucket triplets (sorted by positive
    id so equal-positive slots are contiguous runs), build scatter indices."""
    in_maps = []
    allr = np.arange(N, dtype=np.int64)
    for core in range(NCORES):
        R, H = core >> 1, core & 1
        inH = (allr >> 8) == H
        ownm = (allr & 3) == R
        own_H = allr[ownm & inH]          # 64
        own_O = allr[ownm & ~inH]         # 64
        non_own_H = allr[~ownm & inH]     # 192
        non_own_O = allr[~ownm & ~inH]    # 192
        perm = np.concatenate([non_own_H, own_H, own_O, non_own_O])
        colpos = np.empty(N, dtype=np.int64)
        colpos[perm] = np.arange(N)
        rows_core = np.concatenate([own_H, own_O])  # partition q -> global row
        qof = np.full(N, -1, dtype=np.int64)
        qof[rows_core] = np.arange(NROW)

        sel = ((a & 3) == R) & ((n >> 8) == H)
        asel, psel, nsel = a[sel], p[sel], n[sel]
        q = qof[asel]
        order = np.lexsort((psel, q))
        qs, ps, ns = q[order], psel[order], nsel[order]
        counts = np.bincount(qs, minlength=NROW)
        starts = np.zeros(NROW, dtype=np.int64)
        starts[1:] = np.cumsum(counts)[:-1]
        slot = np.arange(len(qs)) - starts[qs]  # slot within row (sorted by p)

        nidx = np.full((NROW, 256), -1, dtype=np.int16)
        nidx[qs, colpos[ns]] = slot.astype(np.int16)
        if len(qs):
            newrun = np.ones(len(qs), dtype=bool)
            newrun[1:] = (qs[1:] != qs[:-1]) | (ps[1:] != ps[:-1])
        else:
            newrun = np.zeros(0, dtype=bool)
        sidx2 = np.full((NROW, 514), -1, dtype=np.int16)
        sidx2[qs[newrun], colpos[ps[newrun]]] = slot[newrun].astype(np.int16)
        has_room = counts < 256
        sidx2[has_room, 512] = counts[has_room].astype(np.int16)

        keep = np.ones((NROW, 256), dtype=np.float16)
        keep[qs[newrun], slot[newrun]] = 0.0
        keep[has_room, np.minimum(counts, 255)[has_room]] = 0.0

        A16 = samples[perm].astype(np.float16)  # [512, 256]
        AT = np.ascontiguousarray(A16.T)  # [256, 512] = (d, col)
        packa = np.concatenate([AT[0:128], AT[128:256]], axis=1)  # [128, 1024]
        packb = np.concatenate([nidx, sidx2, keep.view(np.int16)], axis=1)
        in_maps.append({"packa": packa, "packb": packb})
    return in_maps


def kernel(samples, targets, anchor_idx, pos_idx, neg_idx, _want_trace=False):
    samples = np.asarray(samples, dtype=np.float32)
    a = np.asarray(anchor_idx).astype(np.int64)
    p = np.asarray(pos_idx).astype(np.int64)
    n = np.asarray(neg_idx).astype(np.int64)
    T = a.shape[0]
    assert samples.shape == (N, D)

    ok = (
        np.all((a >= 0) & (a < N) & (p >= 0) & (p < N) & (n >= 0) & (n < N))
        and len(np.unique(a * N + n)) == T
    )
    if not ok:
        raise NotImplementedError("inputs violate mined-triplet structure")

    nc = _get_program()
    in_maps = _shard_inputs(samples, a, p, n)
    res = run_bass_kernel_spmd(nc, in_maps, list(range(NCORES)), trace=_want_trace)
    total = sum(
        float(res.results[c]["out"].astype(np.float64).sum()) for c in range(NCORES)
    )
    loss = np.float32(total / T)
    if _want_trace:
        return loss, res
    return loss
